# revision 1
# baseline (speedup 1.0000x reference)
"""Trainium2 Bass kernel for nn_Pooling_Layer (GNN message-passing pooling).

Math (per batch element b):
    x = in_pc_pad[b] @ weight_res.T               # (N+1, 64) -> (N+1, 128) projection
    w = |p_neighbors| * mask; w /= w.sum(-1)+1e-8 # (P, 32) pooling weights
    out[b, p] = sum_m w[p, m] * x[id[p, m]]       # gather + weighted pool

We reorder: pool first in C_IN=64 space (gather is half the bytes), then
project pooled (P, 64) @ weight_res.T once.  Normalization is applied after
pooling (divide by the same denom).

Sharding: data-parallel over batch B=8, one NeuronCore per batch element.
p_neighbors / mask / ids / weight_res are replicated (host-prepped layouts).

Gather: dma_gather needs int16 indices; ids reach 40000, so we gather 512B
row-PAIRS with idx = id >> 1 (max 20000, int16-safe) and fold the even/odd
row selection into the pooling weights (weight goes to the correct half,
zero to the other).  Pooling itself runs on the TensorEngine as tiny
block-diagonal stationary matmuls (lhsT is (128, 4): 4 points x 32 slots
contract over the 128 partitions), accumulating even-half and odd-half
matmuls into the same PSUM region.
"""

import numpy as np

import concourse.bass as bass
import concourse.mybir as mybir
import concourse.tile as tile
from concourse import bacc, library_config
from concourse.bass_utils import run_bass_kernel_spmd

F32 = mybir.dt.float32
I16 = mybir.dt.int16

MAXN = 32
CIN = 64
COUT = 128


class Params:
    def __init__(self, pts, npairs, n_cores, bd_chunk_tiles=10, proj_tiles=4):
        self.pts = pts                      # real output points
        self.nt = (pts + 127) // 128        # 128-point tiles
        self.pts_pad = self.nt * 128
        self.npairs = npairs                # rows in the (npairs, 128) pair table
        self.n_cores = n_cores
        self.bd_chunk_tiles = bd_chunk_tiles
        self.proj_tiles = proj_tiles


def build_nc(p: Params):
    nc = bacc.Bacc(
        "TRN2",
        target_bir_lowering=False,
        debug=False,
        num_devices=p.n_cores,
        num_swdge_queues=4,
    )
    x = nc.dram_tensor("x", [p.npairs, 128], F32, kind="ExternalInput")
    idxw = nc.dram_tensor("idxw", [128, p.nt * 256], I16, kind="ExternalInput")
    pnT = nc.dram_tensor("pnT", [128, p.pts_pad], F32, kind="ExternalInput")
    maskT = nc.dram_tensor("maskT", [128, p.pts_pad], F32, kind="ExternalInput")
    parT = nc.dram_tensor("parT", [128, p.pts_pad], F32, kind="ExternalInput")
    pnN = nc.dram_tensor("pnN", [p.pts_pad, MAXN], F32, kind="ExternalInput")
    maskN = nc.dram_tensor("maskN", [p.pts_pad, MAXN], F32, kind="ExternalInput")
    wres = nc.dram_tensor("wres", [COUT, CIN], F32, kind="ExternalInput")
    ident = nc.dram_tensor("ident", [128, 128], F32, kind="ExternalInput")
    out = nc.dram_tensor("out", [p.pts, COUT], F32, kind="ExternalOutput")

    NT = p.nt
    BDT = p.bd_chunk_tiles
    PJT = p.proj_tiles

    with tile.TileContext(nc) as tc:
        with (
            tc.tile_pool(name="const", bufs=1) as constp,
            tc.tile_pool(name="prep", bufs=1) as prep,
            tc.tile_pool(name="bd", bufs=2) as bdp,
            tc.tile_pool(name="wtmp", bufs=1) as wtmp,
            tc.tile_pool(name="gather", bufs=3) as gp,
            tc.tile_pool(name="idx", bufs=3) as idxp,
            tc.tile_pool(name="work", bufs=2) as wk,
            tc.tile_pool(name="ps4", bufs=2, space="PSUM") as ps4,
            tc.tile_pool(name="psT", bufs=1, space="PSUM") as psT,
            tc.tile_pool(name="psP", bufs=1, space="PSUM") as psP,
            tc.tile_pool(name="psB", bufs=2, space="PSUM") as psB,
        ):
            nc.gpsimd.load_library(library_config.mlp)

            # ---- constants ----
            identity = constp.tile([128, 128], F32)
            nc.sync.dma_start(out=identity[:], in_=ident[:])
            wres_sb = constp.tile([COUT, CIN], F32)
            nc.sync.dma_start(out=wres_sb[:], in_=wres[:])
            psw = psT.tile([CIN, COUT], F32, tag="psTt")
            nc.tensor.transpose(out=psw[:], in_=wres_sb[:], identity=identity[:])
            wresT = constp.tile([CIN, COUT], F32)  # [i, o] = wres[o, i]
            nc.vector.tensor_copy(out=wresT[:], in_=psw[:])

            # ---- denominators: recip[pt%128, pt//128] = 1/(sum_m |pn|*mask + 1e-8)
            prodN = prep.tile([128, NT * MAXN], F32)
            nc.sync.dma_start(
                out=prodN[:].rearrange("p (t m) -> p t m", m=MAXN),
                in_=pnN[:].rearrange("(t p) m -> p t m", p=128),
            )
            maskN_sb = prep.tile([128, NT * MAXN], F32)
            nc.sync.dma_start(
                out=maskN_sb[:].rearrange("p (t m) -> p t m", m=MAXN),
                in_=maskN[:].rearrange("(t p) m -> p t m", p=128),
            )
            nc.vector.tensor_tensor(
                out=prodN[:], in0=prodN[:], in1=maskN_sb[:], op=mybir.AluOpType.mult
            )
            denom = constp.tile([128, NT], F32)
            nc.vector.tensor_reduce(
                out=denom[:],
                in_=prodN[:].rearrange("p (t m) -> p t m", m=MAXN),
                op=mybir.AluOpType.add,
                axis=mybir.AxisListType.X,
                apply_absolute_value=True,
            )
            nc.vector.tensor_scalar_add(denom[:], denom[:], 1e-8)
            recip = constp.tile([128, NT], F32)
            nc.vector.reciprocal(out=recip[:], in_=denom[:])

            # ---- main loop ----
            n_chunks = (NT + BDT - 1) // BDT
            bd_cur = None
            poolT_chunk = None
            for t in range(NT):
                c, t_rel = t // BDT, t % BDT
                if t_rel == 0:
                    # ---- build block-diag weight chunk for tiles [c*BDT, ...)
                    ctiles = min(BDT, NT - c * BDT)
                    cpts = ctiles * 128          # points in this chunk
                    cgrp = ctiles * 32           # 4-point groups in this chunk
                    lo = c * BDT * 128
                    pnTc = wtmp.tile([128, BDT * 128], F32, tag="pnTc")
                    maskTc = wtmp.tile([128, BDT * 128], F32, tag="maskTc")
                    parTc = wtmp.tile([128, BDT * 128], F32, tag="parTc")
                    nc.sync.dma_start(out=pnTc[:, :cpts], in_=pnT[:, lo : lo + cpts])
                    nc.sync.dma_start(
                        out=maskTc[:, :cpts], in_=maskT[:, lo : lo + cpts]
                    )
                    nc.sync.dma_start(out=parTc[:, :cpts], in_=parT[:, lo : lo + cpts])
                    wabs = wtmp.tile([128, BDT * 128], F32, tag="wabs")
                    wpar = wtmp.tile([128, BDT * 128], F32, tag="wpar")
                    # wabs = |pn| * mask ; wpar = wabs * parity ; wsel0 = wabs - wpar
                    nc.scalar.activation(
                        out=wabs[:, :cpts],
                        in_=pnTc[:, :cpts],
                        func=mybir.ActivationFunctionType.Abs,
                    )
                    nc.vector.tensor_tensor(
                        out=wabs[:, :cpts],
                        in0=wabs[:, :cpts],
                        in1=maskTc[:, :cpts],
                        op=mybir.AluOpType.mult,
                    )
                    nc.vector.tensor_tensor(
                        out=wpar[:, :cpts],
                        in0=wabs[:, :cpts],
                        in1=parTc[:, :cpts],
                        op=mybir.AluOpType.mult,
                    )
                    nc.vector.tensor_tensor(
                        out=wabs[:, :cpts],  # becomes wsel0 (even-half weights)
                        in0=wabs[:, :cpts],
                        in1=wpar[:, :cpts],
                        op=mybir.AluOpType.subtract,
                    )
                    bd_cur = bdp.tile([128, BDT * 32 * 8], F32, tag="bd")
                    nc.vector.memset(bd_cur[:], 0.0)
                    # scatter weights onto block-diagonal positions:
                    # partition j = 32q + m; point pt = 128(t0+T) + 32q + gl;
                    # bd[32q + m, (T*32 + gl)*8 + 4h + q] = wsel_h[pt, m]
                    for q in range(4):
                        for h, src in ((0, wabs), (1, wpar)):
                            src_ap = src[32 * q : 32 * q + 32, :cpts].rearrange(
                                "p (T r) -> p T r", r=128
                            )[:, :, 32 * q : 32 * q + 32]
                            dst_ap = bd_cur[
                                32 * q : 32 * q + 32, : cgrp * 8
                            ].rearrange("p (T g e) -> p T g e", g=32, e=8)[
                                :, :, :, 4 * h + q
                            ]
                            nc.vector.tensor_copy(out=dst_ap, in_=src_ap)

                # ---- gather tile t: 4096 pair-rows ----
                idx_sb = idxp.tile([128, 256], I16)
                nc.sync.dma_start(out=idx_sb[:], in_=idxw[:, t * 256 : (t + 1) * 256])
                # 4 x 1024-idx gathers (SWDGE ring caps ~127 descriptors per
                # lane per call) striped across the 4 SWDGE queues so the
                # Q7 descriptor generation runs on all four core pairs.
                g = gp.tile([128, 32 * 128], F32, tag="g")
                for k in range(4):
                    nc.gpsimd.dma_gather(
                        g[:, k * 1024 : (k + 1) * 1024].rearrange(
                            "p (t e) -> p t e", e=128
                        ),
                        x[:],
                        idx_sb[:, k * 64 : (k + 1) * 64],
                        1024,
                        1024,
                        128,
                        queue_num=k,
                    )

                # ---- pooling: 64 matmuls -> psum (4, 64) per 4-point group ----
                pooled4 = wk.tile([4, 2048], F32, tag="pooled4")
                for half in range(2):
                    ps = ps4.tile([4, 1024], F32)
                    for gl in range(16):
                        grp = 16 * half + gl
                        base = (t_rel * 32 + grp) * 8
                        for h in range(2):
                            nc.tensor.matmul(
                                out=ps[:, gl * 64 : (gl + 1) * 64],
                                lhsT=bd_cur[:, base + 4 * h : base + 4 * h + 4],
                                rhs=g[
                                    :, grp * 128 + 64 * h : grp * 128 + 64 * h + 64
                                ],
                                start=(h == 0),
                                stop=(h == 1),
                            )
                    nc.scalar.copy(
                        out=pooled4[:, half * 1024 : (half + 1) * 1024], in_=ps[:]
                    )

                # ---- repack (4, 32, 64) -> (128, 64) so partition = point ----
                # pooled row r = 32 q + gl matches the element order of
                # pooled4 (q-partition outer, gl then channel inner).
                pooled = wk.tile([128, 64], F32, tag="pooled")
                nc.sync.dma_start(
                    out=pooled[:],
                    in_=pooled4[:].rearrange("q (g e) -> q g e", e=64),
                )
                # normalize by 1/denom (per-partition scalar)
                nc.vector.tensor_scalar_mul(pooled[:], pooled[:], recip[:, t : t + 1])

                # ---- transpose pooled -> poolT[(i), (pt)] ----
                if t % PJT == 0:
                    poolT_chunk = wk.tile([CIN, PJT * 128], F32, tag="poolT")
                psTt = psT.tile([CIN, 128], F32, tag="psTt")
                nc.tensor.transpose(out=psTt[:], in_=pooled[:], identity=identity[:])
                j = t % PJT
                nc.vector.tensor_copy(
                    out=poolT_chunk[:, j * 128 : (j + 1) * 128], in_=psTt[:]
                )

                # ---- projection + transpose back + store, every PJT tiles ----
                if t % PJT == PJT - 1 or t == NT - 1:
                    nb = (t % PJT) + 1  # tiles in this projection chunk
                    psp = psP.tile([COUT, PJT * 128], F32)
                    nc.tensor.matmul(
                        out=psp[:, : nb * 128],
                        lhsT=wresT[:],
                        rhs=poolT_chunk[:, : nb * 128],
                        start=True,
                        stop=True,
                    )
                    outT = wk.tile([COUT, PJT * 128], F32, tag="outT")
                    nc.scalar.copy(out=outT[:, : nb * 128], in_=psp[:, : nb * 128])
                    t0 = t - nb + 1
                    for k in range(nb):
                        psb = psB.tile([128, COUT], F32)
                        nc.tensor.transpose(
                            out=psb[:],
                            in_=outT[:, k * 128 : (k + 1) * 128],
                            identity=identity[:],
                        )
                        outP = wk.tile([128, COUT], F32, tag="outP")
                        nc.vector.tensor_copy(out=outP[:], in_=psb[:])
                        row0 = (t0 + k) * 128
                        nrows = min(128, p.pts - row0)
                        if nrows > 0:
                            nc.sync.dma_start(
                                out=out[row0 : row0 + nrows, :],
                                in_=outP[:nrows, :],
                            )
    nc.compile()
    return nc


def host_prep(p: Params, in_pc_pad, ids, mask, pn, wres):
    """Build per-core input maps.  All host work is sharding / index prep /
    layout marshalling — no model FLOPs."""
    B = in_pc_pad.shape[0]
    pts, pts_pad, nt = p.pts, p.pts_pad, p.nt

    ids = np.asarray(ids).astype(np.int64)
    pn = np.asarray(pn, dtype=np.float32)
    mask = np.asarray(mask, dtype=np.float32)
    wres = np.asarray(wres, dtype=np.float32)

    def pad_pts(a, dtype):
        out = np.zeros((pts_pad, MAXN), dtype=dtype)
        out[:pts] = a
        return out

    idx16 = pad_pts(ids >> 1, np.int16)          # pair index, int16-safe
    par = pad_pts((ids & 1).astype(np.float32), np.float32)
    pn_p = pad_pts(pn, np.float32)
    mask_p = pad_pts(mask, np.float32)

    # gather index stream: per tile t, i_local = gl*128 + q*32 + m,
    # point pt = 128 t + 32 q + gl
    flat = idx16.reshape(nt, 4, 32, MAXN).transpose(0, 2, 1, 3).reshape(nt, 4096)
    # wrapped-16 layout: idx i at [i % 16, i // 16], replicated to 128 parts
    idx_w = np.zeros((128, nt * 256), np.int16)
    for t in range(nt):
        blk = flat[t].reshape(256, 16).T  # (16, 256)
        idx_w[:, t * 256 : (t + 1) * 256] = np.tile(blk, (8, 1))

    pnT = np.ascontiguousarray(np.tile(pn_p.T, (4, 1)))      # (128, pts_pad)
    maskT = np.ascontiguousarray(np.tile(mask_p.T, (4, 1)))
    parT = np.ascontiguousarray(np.tile(par.T, (4, 1)))
    ident = np.eye(128, dtype=np.float32)

    shared = {
        "idxw": idx_w,
        "pnT": pnT,
        "maskT": maskT,
        "parT": parT,
        "pnN": pn_p,
        "maskN": mask_p,
        "wres": wres,
        "ident": ident,
    }
    in_maps = []
    for b in range(B):
        xb = np.concatenate(
            [np.asarray(in_pc_pad[b], np.float32), np.zeros((1, CIN), np.float32)], 0
        )
        xb = np.ascontiguousarray(xb.reshape(p.npairs, 128))
        in_maps.append({"x": xb, **shared})
    return in_maps


_NC_CACHE = {}


def _get_nc(p: Params):
    key = (p.pts, p.npairs, p.n_cores, p.bd_chunk_tiles, p.proj_tiles)
    if key not in _NC_CACHE:
        _NC_CACHE[key] = build_nc(p)
    return _NC_CACHE[key]


# ---------------------------------------------------------------------------
# v2: batch-interleaved table, points-sharded across cores.
#
# The table is laid out as (npairs, B*128): one 4 KB gather descriptor
# fetches a row-pair for ALL B batch elements at once, cutting SWDGE
# descriptor-generation work (the v1 bottleneck) by 8x.  Each core owns
# pts/B points for all batches; outputs are reassembled on the host.
# ---------------------------------------------------------------------------


class ParamsIL:
    def __init__(self, pts, npairs, n_cores, proj_tiles=4):
        self.pts = pts                        # total points (split over cores)
        self.n_cores = n_cores
        self.B = n_cores                      # batch size == cores
        self.cpts = pts // n_cores            # points per core (1250)
        self.cpts_pad = ((self.cpts + 15) // 16) * 16  # row-tile = 16 pts
        self.ngrp = self.cpts_pad // 4        # 4-point groups per core
        assert self.B == 8, "IL layout assumes 8 batches (16 pts x 8 b = 128 rows)"
        self.nrt = self.cpts_pad // 16        # 128-row tiles (rows = (pt, b))
        self.npairs = npairs
        self.proj_tiles = proj_tiles


def build_nc_il(p: ParamsIL):
    B = p.B
    EW = B * 128                              # interleaved elem width (f32)
    nc = bacc.Bacc(
        "TRN2",
        target_bir_lowering=False,
        debug=False,
        num_devices=p.n_cores,
        num_swdge_queues=4,
    )
    xi = nc.dram_tensor("xi", [p.npairs, EW], F32, kind="ExternalInput")
    idxw = nc.dram_tensor("idxw", [128, p.ngrp * 8], I16, kind="ExternalInput")
    pnT = nc.dram_tensor("pnT", [128, p.cpts_pad], F32, kind="ExternalInput")
    maskT = nc.dram_tensor("maskT", [128, p.cpts_pad], F32, kind="ExternalInput")
    parT = nc.dram_tensor("parT", [128, p.cpts_pad], F32, kind="ExternalInput")
    pnN8 = nc.dram_tensor("pnN8", [p.cpts_pad * B, MAXN], F32, kind="ExternalInput")
    maskN8 = nc.dram_tensor(
        "maskN8", [p.cpts_pad * B, MAXN], F32, kind="ExternalInput"
    )
    wres = nc.dram_tensor("wres", [COUT, CIN], F32, kind="ExternalInput")
    ident = nc.dram_tensor("ident", [128, 128], F32, kind="ExternalInput")
    nrows = p.cpts * B                        # valid output rows (pt-major, b minor)
    out = nc.dram_tensor("out", [nrows, COUT], F32, kind="ExternalOutput")

    NRT = p.nrt
    PJT = p.proj_tiles

    with tile.TileContext(nc) as tc:
        with (
            tc.tile_pool(name="const", bufs=1) as constp,
            tc.tile_pool(name="prep", bufs=1) as prep,
            tc.tile_pool(name="gather", bufs=6) as gp,
            tc.tile_pool(name="work", bufs=2) as wk,
            tc.tile_pool(name="p4", bufs=4) as p4p,
            tc.tile_pool(name="ps4", bufs=4, space="PSUM") as ps4,
            tc.tile_pool(name="psT", bufs=1, space="PSUM") as psT,
            tc.tile_pool(name="psP", bufs=1, space="PSUM") as psP,
            tc.tile_pool(name="psB", bufs=2, space="PSUM") as psB,
        ):
            nc.gpsimd.load_library(library_config.mlp)

            # ---- constants ----
            identity = constp.tile([128, 128], F32)
            nc.sync.dma_start(out=identity[:], in_=ident[:])
            wres_sb = constp.tile([COUT, CIN], F32)
            nc.sync.dma_start(out=wres_sb[:], in_=wres[:])
            psw = psT.tile([CIN, COUT], F32, tag="psTt")
            nc.tensor.transpose(out=psw[:], in_=wres_sb[:], identity=identity[:])
            wresT = constp.tile([CIN, COUT], F32)
            nc.vector.tensor_copy(out=wresT[:], in_=psw[:])

            idx_sb = constp.tile([128, p.ngrp * 8], I16)
            nc.sync.dma_start(out=idx_sb[:], in_=idxw[:])

            # ---- per-row reciprocal denominators (rows = (pt, b)) ----
            prodN = prep.tile([128, NRT * MAXN], F32)
            nc.sync.dma_start(
                out=prodN[:].rearrange("p (t m) -> p t m", m=MAXN),
                in_=pnN8[:].rearrange("(t p) m -> p t m", p=128),
            )
            maskN_sb = prep.tile([128, NRT * MAXN], F32)
            nc.sync.dma_start(
                out=maskN_sb[:].rearrange("p (t m) -> p t m", m=MAXN),
                in_=maskN8[:].rearrange("(t p) m -> p t m", p=128),
            )
            nc.vector.tensor_tensor(
                out=prodN[:], in0=prodN[:], in1=maskN_sb[:], op=mybir.AluOpType.mult
            )
            denom = constp.tile([128, NRT], F32)
            nc.vector.tensor_reduce(
                out=denom[:],
                in_=prodN[:].rearrange("p (t m) -> p t m", m=MAXN),
                op=mybir.AluOpType.add,
                axis=mybir.AxisListType.X,
                apply_absolute_value=True,
            )
            nc.vector.tensor_scalar_add(denom[:], denom[:], 1e-8)
            recip = constp.tile([128, NRT], F32)
            nc.vector.reciprocal(out=recip[:], in_=denom[:])

            # ---- block-diagonal pooling weights, built once ----
            pnTc = prep.tile([128, p.cpts_pad], F32)
            maskTc = prep.tile([128, p.cpts_pad], F32)
            parTc = prep.tile([128, p.cpts_pad], F32)
            nc.sync.dma_start(out=pnTc[:], in_=pnT[:])
            nc.sync.dma_start(out=maskTc[:], in_=maskT[:])
            nc.sync.dma_start(out=parTc[:], in_=parT[:])
            wabs = prep.tile([128, p.cpts_pad], F32)
            wpar = prep.tile([128, p.cpts_pad], F32)
            nc.scalar.activation(
                out=wabs[:], in_=pnTc[:], func=mybir.ActivationFunctionType.Abs
            )
            nc.vector.tensor_tensor(
                out=wabs[:], in0=wabs[:], in1=maskTc[:], op=mybir.AluOpType.mult
            )
            nc.vector.tensor_tensor(
                out=wpar[:], in0=wabs[:], in1=parTc[:], op=mybir.AluOpType.mult
            )
            nc.vector.tensor_tensor(
                out=wabs[:], in0=wabs[:], in1=wpar[:], op=mybir.AluOpType.subtract
            )
            bd = constp.tile([128, p.ngrp * 8], F32)
            nc.vector.memset(bd[:], 0.0)
            for q in range(4):
                for h, src in ((0, wabs), (1, wpar)):
                    src_ap = src[32 * q : 32 * q + 32, :].rearrange(
                        "p (g four) -> p g four", four=4
                    )[:, :, q]
                    dst_ap = bd[32 * q : 32 * q + 32, :].rearrange(
                        "p (g e) -> p g e", e=8
                    )[:, :, 4 * h + q]
                    nc.vector.tensor_copy(out=dst_ap, in_=src_ap)

            # ---- main loop: 2 groups per gather call ----
            ncall = p.ngrp // 2
            poolT_chunk = None
            for T in range(NRT):          # row-tile = 4 groups = 16 pts
                pooled = wk.tile([128, CIN], F32, tag="pooled")
                for half in range(2):     # one gather call = 2 groups
                    call = T * 2 + half
                    g = gp.tile([128, 2 * EW], F32, tag="g")
                    nc.gpsimd.dma_gather(
                        g[:].rearrange("p (t e) -> p t e", e=EW),
                        xi[:],
                        idx_sb[:, call * 16 : (call + 1) * 16],
                        256,
                        256,
                        EW,
                        queue_num=call % 4,
                    )
                    for gs in range(2):   # groups within the call
                        gl = half * 2 + gs            # group-in-tile 0..3
                        grp = T * 4 + gl              # global group
                        ps = ps4.tile([4, B * CIN], F32)
                        for h in range(2):
                            nc.tensor.matmul(
                                out=ps[:],
                                lhsT=bd[:, grp * 8 + 4 * h : grp * 8 + 4 * h + 4],
                                rhs=g[:, gs * EW : (gs + 1) * EW].rearrange(
                                    "p (b e) -> p b e", e=128
                                )[:, :, 64 * h : 64 * h + 64],
                                start=(h == 0),
                                stop=(h == 1),
                            )
                        pooled4 = p4p.tile([4, B * CIN], F32, tag="pooled4")
                        nc.scalar.copy(out=pooled4[:], in_=ps[:])
                        # repack rows: r = 32 gl + 8 q + b
                        nc.sync.dma_start(
                            out=pooled[32 * gl : 32 * gl + 32, :],
                            in_=pooled4[:].rearrange("q (b e) -> q b e", e=CIN),
                        )
                nc.vector.tensor_scalar_mul(pooled[:], pooled[:], recip[:, T : T + 1])

                if T % PJT == 0:
                    poolT_chunk = wk.tile([CIN, PJT * 128], F32, tag="poolT")
                psTt = psT.tile([CIN, 128], F32, tag="psTt")
                nc.tensor.transpose(out=psTt[:], in_=pooled[:], identity=identity[:])
                j = T % PJT
                nc.vector.tensor_copy(
                    out=poolT_chunk[:, j * 128 : (j + 1) * 128], in_=psTt[:]
                )

                if T % PJT == PJT - 1 or T == NRT - 1:
                    nb = (T % PJT) + 1
                    psp = psP.tile([COUT, PJT * 128], F32)
                    nc.tensor.matmul(
                        out=psp[:, : nb * 128],
                        lhsT=wresT[:],
                        rhs=poolT_chunk[:, : nb * 128],
                        start=True,
                        stop=True,
                    )
                    outT = wk.tile([COUT, PJT * 128], F32, tag="outT")
                    nc.scalar.copy(out=outT[:, : nb * 128], in_=psp[:, : nb * 128])
                    t0 = T - nb + 1
                    for k in range(nb):
                        psb = psB.tile([128, COUT], F32)
                        nc.tensor.transpose(
                            out=psb[:],
                            in_=outT[:, k * 128 : (k + 1) * 128],
                            identity=identity[:],
                        )
                        outP = wk.tile([128, COUT], F32, tag="outP")
                        nc.vector.tensor_copy(out=outP[:], in_=psb[:])
                        row0 = (t0 + k) * 128
                        nr = min(128, nrows - row0)
                        if nr > 0:
                            nc.sync.dma_start(
                                out=out[row0 : row0 + nr, :], in_=outP[:nr, :]
                            )
    nc.compile()
    return nc


def host_prep_il(p: ParamsIL, in_pc_pad, ids, mask, pn, wres):
    B = p.B
    ids = np.asarray(ids).astype(np.int64)
    pn = np.asarray(pn, dtype=np.float32)
    mask = np.asarray(mask, dtype=np.float32)
    wres = np.asarray(wres, dtype=np.float32)
    in_pc_pad = np.asarray(in_pc_pad, dtype=np.float32)

    # interleaved pair table (npairs, B*128): pair k, batch b, 128 channels
    xpad = np.concatenate(
        [in_pc_pad, np.zeros((B, 1, CIN), np.float32)], axis=1
    ).reshape(B, p.npairs, 128)
    xi = np.ascontiguousarray(xpad.transpose(1, 0, 2).reshape(p.npairs, B * 128))

    idx16 = (ids >> 1).astype(np.int16)           # (pts, 32)
    par = (ids & 1).astype(np.float32)
    ident = np.eye(128, dtype=np.float32)

    in_maps = []
    for c in range(p.n_cores):
        lo = c * p.cpts
        sl = slice(lo, lo + p.cpts)

        def pad_pts(a, dtype):
            o = np.zeros((p.cpts_pad, MAXN), dtype=dtype)
            o[: p.cpts] = a[sl]
            return o

        idx_c = pad_pts(idx16, np.int16)
        par_c = pad_pts(par, np.float32)
        pn_c = pad_pts(pn, np.float32)
        mask_c = pad_pts(mask, np.float32)

        # gather stream: per call (256 idx = 2 groups): i = gs*128 + q*32 + m,
        # pt = 4*grp + q
        flat = idx_c.reshape(p.ngrp * 128)        # [grp, q, m] order
        idx_w = np.zeros((128, p.ngrp * 8), np.int16)
        for call in range(p.ngrp // 2):
            blk = flat[call * 256 : (call + 1) * 256].reshape(16, 16).T
            idx_w[:, call * 16 : (call + 1) * 16] = np.tile(blk, (8, 1))

        pnT = np.ascontiguousarray(np.tile(pn_c.T, (4, 1)))
        maskT = np.ascontiguousarray(np.tile(mask_c.T, (4, 1)))
        parT = np.ascontiguousarray(np.tile(par_c.T, (4, 1)))
        pnN8 = np.ascontiguousarray(np.repeat(pn_c, B, axis=0))
        maskN8 = np.ascontiguousarray(np.repeat(mask_c, B, axis=0))
        in_maps.append(
            {
                "xi": xi,
                "idxw": idx_w,
                "pnT": pnT,
                "maskT": maskT,
                "parT": parT,
                "pnN8": pnN8,
                "maskN8": maskN8,
                "wres": wres,
                "ident": ident,
            }
        )
    return in_maps


def assemble_il(p: ParamsIL, results):
    B = p.B
    out = np.empty((B, p.pts, COUT), np.float32)
    for c in range(p.n_cores):
        got = results[c]["out"].reshape(p.cpts, B, COUT)
        out[:, c * p.cpts : (c + 1) * p.cpts, :] = got.transpose(1, 0, 2)
    return out


def kernel(in_pc_pad, neighbor_id_lstlst, neighbor_mask_lst, p_neighbors, weight_res):
    in_pc_pad = np.asarray(in_pc_pad)
    B = in_pc_pad.shape[0]
    p = ParamsIL(pts=10000, npairs=20001, n_cores=B)
    in_maps = host_prep_il(
        p, in_pc_pad, neighbor_id_lstlst, neighbor_mask_lst, p_neighbors, weight_res
    )
    key = ("il", p.pts, p.npairs, p.n_cores)
    if key not in _NC_CACHE:
        _NC_CACHE[key] = build_nc_il(p)
    nc = _NC_CACHE[key]
    res = run_bass_kernel_spmd(nc, in_maps, core_ids=list(range(B)))
    return assemble_il(p, res.results)



# revision 11
# speedup vs baseline: 2.7465x; 2.7465x over previous
"""Trainium2 Bass kernel for nn_Pooling_Layer (GNN message-passing pooling).

Math (per batch element b):
    x = in_pc_pad[b] @ weight_res.T               # (N+1, 64) -> (N+1, 128) projection
    w = |p_neighbors| * mask; w /= w.sum(-1)+1e-8 # (P, 32) pooling weights
    out[b, p] = sum_m w[p, m] * x[id[p, m]]       # gather + weighted pool

We reorder: pool first in C_IN=64 space (gather is half the bytes), then
project pooled (P, 64) @ weight_res.T.  Normalization (divide by the weight
sum) is folded into the PSUM->SBUF copy after the projection.

Sharding: points are sharded across the 8 cores (1250 points each); every
core handles ALL batches for its points.  The gather table holds row PAIRS,
batch-interleaved, in bf16: xi[k] = [row 2k: b0..b7 x 64ch | row 2k+1:
b0..b7 x 64ch] (2KB rows).  Pairs keep the SWDGE gather indices int16-safe
(idx = id >> 1 <= 20000); one descriptor serves all 8 batches at a
DMA-efficient 2KB.  bf16 halves HBM gather traffic vs f32; the tolerance
(2e-2) dwarfs bf16 rounding (~0.5%).

Pooling runs on the TensorEngine: per 128-point tile, 64 accumulating bf16
matmuls (32 windows x even/odd half) into one (128 pts, 8b*64ch) PSUM bank.
lhsT is a block-diagonal weight matrix with a FIXED sparsity structure:
window w (slots = partitions: slot 32q+m = neighbor m of point 4w+q) puts
weight at [32q+m, 4w+q].  Even-half weights are |pn|*mask*(1-parity), odd
|pn|*mask*parity, so the wrong half of each gathered pair contributes 0.
The nonzero positions are identical for every tile, so the bd buffers are
zeroed once and only the values are rewritten per tile (tiny strided
copies).

Then per tile: 4 PE transposes (128pts, 2 batches*64ch) -> (128ch, 128pts),
8 projection matmuls lhsT=pooled^T (64,128) rhs=weight_res^T (64,128), and
the per-point 1/denom scale on the PSUM->SBUF copy.  Output is bf16,
upcast and re-assembled on the host.
"""

import numpy as np
import ml_dtypes

import concourse.bass as bass
import concourse.mybir as mybir
import concourse.tile as tile
from concourse import bacc, library_config
from concourse.bass_utils import run_bass_kernel_spmd

F32 = mybir.dt.float32
BF16 = mybir.dt.bfloat16
I16 = mybir.dt.int16

MAXN = 32
CIN = 64
COUT = 128
B = 8
IN_ROWS = 40001          # in_pc_pad rows (incl. pad row)
NPAIRS = 20001           # row pairs (rows padded to 40002)
EW = B * CIN             # interleaved single-row width (elements) = 512
PEW = 2 * EW             # pair-row width = 1024 elements (2KB bf16)
PTS = 10000
NWIN = 32                # windows (4-point groups) per 128-point tile
CHW = 8                  # windows per gather call (1024 idx)
NCALL = NWIN // CHW      # gather calls per tile


class Params:
    def __init__(self, pts=PTS, n_cores=8):
        self.pts = pts
        self.n_cores = n_cores
        self.cpts = pts // n_cores            # points per core (1250)
        self.ntl = (self.cpts + 127) // 128   # 128-point tiles per core (10)
        self.cpts_pad = self.ntl * 128        # 1280


def build_nc(p: Params):
    nc = bacc.Bacc(
        "TRN2",
        target_bir_lowering=False,
        debug=False,
        num_devices=p.n_cores,
        num_swdge_queues=4,
    )
    NTL = p.ntl
    xi = nc.dram_tensor("xi", [NPAIRS, PEW], BF16, kind="ExternalInput")
    idxw = nc.dram_tensor("idxw", [128, NTL * NCALL * 64], I16, kind="ExternalInput")
    pnT = nc.dram_tensor("pnT", [128, p.cpts_pad], F32, kind="ExternalInput")
    maskT = nc.dram_tensor("maskT", [128, p.cpts_pad], F32, kind="ExternalInput")
    parT = nc.dram_tensor("parT", [128, p.cpts_pad], F32, kind="ExternalInput")
    pnN = nc.dram_tensor("pnN", [p.cpts_pad, MAXN], F32, kind="ExternalInput")
    maskN = nc.dram_tensor("maskN", [p.cpts_pad, MAXN], F32, kind="ExternalInput")
    wres = nc.dram_tensor("wres", [COUT, CIN], F32, kind="ExternalInput")
    ident = nc.dram_tensor("ident", [128, 128], F32, kind="ExternalInput")
    out = nc.dram_tensor("out", [B * p.cpts_pad, COUT], BF16, kind="ExternalOutput")

    with tile.TileContext(nc) as tc:
        with (
            tc.tile_pool(name="const", bufs=1) as constp,
            tc.tile_pool(name="prep", bufs=1) as prep,
            tc.tile_pool(name="gather", bufs=6) as gp,
            tc.tile_pool(name="work", bufs=2) as wk,
            tc.tile_pool(name="psP", bufs=2, space="PSUM") as psP,
            tc.tile_pool(name="psT", bufs=2, space="PSUM") as psT,
            tc.tile_pool(name="psO", bufs=2, space="PSUM") as psO,
        ):
            nc.gpsimd.load_library(library_config.mlp)

            # ---- constants ----
            identity = constp.tile([128, 128], F32)
            nc.sync.dma_start(out=identity[:], in_=ident[:])
            wres_sb = constp.tile([COUT, CIN], F32)
            nc.sync.dma_start(out=wres_sb[:], in_=wres[:])
            psw = psT.tile([CIN, COUT], F32, tag="psTt")
            nc.tensor.transpose(out=psw[:], in_=wres_sb[:], identity=identity[:])
            # [i, o] = wres[o, i], replicated into both 64-partition halves so
            # the projection matmul's rhs base partition matches lhsT's
            wresTb = constp.tile([128, COUT], BF16)
            nc.vector.tensor_copy(out=wresTb[0:CIN, :], in_=psw[:])
            nc.vector.tensor_copy(out=wresTb[CIN : 2 * CIN, :], in_=psw[:])

            idx_sb = constp.tile([128, NTL * NCALL * 64], I16)
            nc.sync.dma_start(out=idx_sb[:], in_=idxw[:])

            # ---- per-point reciprocal denominators: recip[p, t] ----
            prodN = prep.tile([128, NTL * MAXN], F32)
            nc.sync.dma_start(
                out=prodN[:].rearrange("p (t m) -> p t m", m=MAXN),
                in_=pnN[:].rearrange("(t p) m -> p t m", p=128),
            )
            maskN_sb = prep.tile([128, NTL * MAXN], F32)
            nc.sync.dma_start(
                out=maskN_sb[:].rearrange("p (t m) -> p t m", m=MAXN),
                in_=maskN[:].rearrange("(t p) m -> p t m", p=128),
            )
            nc.vector.tensor_tensor(
                out=prodN[:], in0=prodN[:], in1=maskN_sb[:], op=mybir.AluOpType.mult
            )
            denom = constp.tile([128, NTL], F32)
            nc.vector.tensor_reduce(
                out=denom[:],
                in_=prodN[:].rearrange("p (t m) -> p t m", m=MAXN),
                op=mybir.AluOpType.add,
                axis=mybir.AxisListType.X,
                apply_absolute_value=True,
            )
            nc.vector.tensor_scalar_add(denom[:], denom[:], 1e-8)
            recip = constp.tile([128, NTL], F32)
            nc.vector.reciprocal(out=recip[:], in_=denom[:])

            # ---- pooling weights in (32q+m, pt) layout ----
            # wsel0 = |pn|*mask*(1-par)   (even half)
            # wsel1 = |pn|*mask*par       (odd half)
            pnT_sb = prep.tile([128, p.cpts_pad], F32)
            maskT_sb = prep.tile([128, p.cpts_pad], F32)
            parT_sb = prep.tile([128, p.cpts_pad], F32)
            nc.sync.dma_start(out=pnT_sb[:], in_=pnT[:])
            nc.sync.dma_start(out=maskT_sb[:], in_=maskT[:])
            nc.sync.dma_start(out=parT_sb[:], in_=parT[:])
            wsel0 = prep.tile([128, p.cpts_pad], F32)
            wsel1 = prep.tile([128, p.cpts_pad], F32)
            nc.scalar.activation(
                out=wsel0[:], in_=pnT_sb[:], func=mybir.ActivationFunctionType.Abs
            )
            nc.vector.tensor_tensor(
                out=wsel0[:], in0=wsel0[:], in1=maskT_sb[:], op=mybir.AluOpType.mult
            )
            nc.vector.tensor_tensor(
                out=wsel1[:], in0=wsel0[:], in1=parT_sb[:], op=mybir.AluOpType.mult
            )
            nc.vector.tensor_tensor(
                out=wsel0[:], in0=wsel0[:], in1=wsel1[:], op=mybir.AluOpType.subtract
            )

            # ---- block-diag weight buffers: fixed sparsity, zeroed once ----
            BDW = NWIN * 132  # 4224: bd[s, 132w + q] == lhsT col 4w+q of window w
            bd_bufs = []      # [t%2][half] ping-pong pairs
            for i in range(2):
                pair = []
                for half in range(2):
                    bdt = constp.tile([128, BDW], BF16, tag=f"bd{i}h{half}")
                    nc.vector.memset(bdt[:], 0.0)
                    pair.append(bdt)
                bd_bufs.append(pair)

            # ---- main loop over 128-point tiles ----
            for t in range(NTL):
                # scatter this tile's weights onto the fixed block-diag slots
                bde, bdo = bd_bufs[t % 2]
                for bd, src in ((bde, wsel0), (bdo, wsel1)):
                    bdv = bd[:].rearrange("p (w c) -> p w c", c=132)
                    sv = src[:, t * 128 : (t + 1) * 128].rearrange(
                        "p (w four) -> p w four", four=4
                    )
                    for q in range(4):
                        nc.vector.tensor_copy(
                            out=bdv[32 * q : 32 * q + 32, :, q],
                            in_=sv[32 * q : 32 * q + 32, :, q],
                        )

                # gather + pool in 4 chunks of 8 windows (1024 idx per call)
                ps = psP.tile([128, EW], F32, tag="ps")
                for c in range(NCALL):
                    g = gp.tile([128, CHW * PEW], BF16, tag="g")
                    call = t * NCALL + c
                    nc.gpsimd.dma_gather(
                        g[:].rearrange("p (v e) -> p v e", e=PEW),
                        xi[:],
                        idx_sb[:, call * 64 : (call + 1) * 64],
                        CHW * 128,
                        CHW * 128,
                        PEW,
                        queue_num=call % 4,
                    )
                    for v in range(CHW):
                        w = c * CHW + v
                        for half, bd in ((0, bde), (1, bdo)):
                            nc.tensor.matmul(
                                out=ps[:],
                                lhsT=bd[:, w * 128 : w * 128 + 128],
                                rhs=g[
                                    :,
                                    v * PEW + half * EW : v * PEW + (half + 1) * EW,
                                ],
                                start=(w == 0 and half == 0),
                                stop=(w == NWIN - 1 and half == 1),
                            )
                pooled = wk.tile([128, EW], F32, tag="pooled")
                nc.scalar.copy(out=pooled[:], in_=ps[:])

                # transpose 2-batch blocks, project, scale by 1/denom, store
                for k in range(4):
                    pst = psT.tile([128, 128], F32, tag="psTt")
                    nc.tensor.transpose(
                        out=pst[:],
                        in_=pooled[:, k * 128 : (k + 1) * 128],
                        identity=identity[:],
                    )
                    poolTb = wk.tile([128, 128], BF16, tag="poolTb")
                    nc.vector.tensor_copy(out=poolTb[:], in_=pst[:])
                    for h in range(2):
                        b = 2 * k + h
                        pso = psO.tile([128, COUT], F32, tag="psO")
                        nc.tensor.matmul(
                            out=pso[:],
                            lhsT=poolTb[64 * h : 64 * h + 64, :],
                            rhs=wresTb[64 * h : 64 * h + 64, :],
                            start=True,
                            stop=True,
                        )
                        outP = wk.tile([128, COUT], BF16, tag="outP")
                        nc.vector.tensor_scalar_mul(
                            outP[:], pso[:], recip[:, t : t + 1]
                        )
                        r0 = b * p.cpts_pad + t * 128
                        nc.sync.dma_start(out=out[r0 : r0 + 128, :], in_=outP[:])
    nc.compile()
    return nc


def host_prep(p: Params, in_pc_pad, ids, mask, pn, wres):
    """Per-core input maps.  Host work is layout marshalling only."""
    ids = np.asarray(ids).astype(np.int64)
    pn = np.asarray(pn, dtype=np.float32)
    mask = np.asarray(mask, dtype=np.float32)
    wres = np.asarray(wres, dtype=np.float32)
    x = np.asarray(in_pc_pad, dtype=np.float32)          # (B, 40001, 64)

    # pair table: xi[k] = [row 2k all batches | row 2k+1 all batches], bf16
    xp = np.concatenate([x, np.zeros((B, 1, CIN), np.float32)], axis=1)
    xi = np.ascontiguousarray(
        xp.transpose(1, 0, 2).reshape(2 * NPAIRS, EW).reshape(NPAIRS, PEW)
    ).astype(ml_dtypes.bfloat16)
    ident = np.eye(128, dtype=np.float32)

    in_maps = []
    for c in range(p.n_cores):
        lo = c * p.cpts

        def pad_pts(a, dtype):
            o = np.zeros((p.cpts_pad, MAXN), dtype=dtype)
            o[: p.cpts] = a[lo : lo + p.cpts]
            return o

        ids_c = pad_pts(ids, np.int64)
        ids_c[p.cpts :] = 2 * (NPAIRS - 1)               # pad points: valid pair
        pn_c = pad_pts(pn, np.float32)
        mask_c = pad_pts(mask, np.float32)
        par_c = (ids_c & 1).astype(np.float32)
        idx16 = (ids_c >> 1).astype(np.int16)

        # gather stream: tile t, window w, slot s=32q+m -> ids_c[t*128+4w+q, m]>>1
        flat = (
            idx16.reshape(p.ntl, NWIN, 4, MAXN)
            .transpose(0, 1, 2, 3)                       # (t, w, q, m)
            .reshape(p.ntl * NWIN * 128)
        )
        # wrapped-16 layout per 1024-idx call: idx i at [i % 16, i // 16]
        ncalls = p.ntl * NCALL
        idx_w = np.zeros((128, ncalls * 64), np.int16)
        for call in range(ncalls):
            blk = flat[call * 1024 : (call + 1) * 1024].reshape(64, 16).T
            idx_w[:, call * 64 : (call + 1) * 64] = np.tile(blk, (8, 1))

        pnT = np.ascontiguousarray(np.tile(pn_c.T, (4, 1)))      # (128, cpts_pad)
        maskT = np.ascontiguousarray(np.tile(mask_c.T, (4, 1)))
        parT = np.ascontiguousarray(np.tile(par_c.T, (4, 1)))
        in_maps.append(
            {
                "xi": xi,
                "idxw": idx_w,
                "pnT": pnT,
                "maskT": maskT,
                "parT": parT,
                "pnN": pn_c,
                "maskN": mask_c,
                "wres": wres,
                "ident": ident,
            }
        )
    return in_maps


def assemble(p: Params, results):
    out = np.empty((B, p.pts, COUT), np.float32)
    for c in range(p.n_cores):
        got = np.asarray(results[c]["out"], dtype=np.float32).reshape(
            B, p.cpts_pad, COUT
        )
        out[:, c * p.cpts : (c + 1) * p.cpts, :] = got[:, : p.cpts, :]
    return out


_NC_CACHE = {}


def get_nc(p: Params):
    key = (p.pts, p.n_cores)
    if key not in _NC_CACHE:
        _NC_CACHE[key] = build_nc(p)
    return _NC_CACHE[key]


def kernel(in_pc_pad, neighbor_id_lstlst, neighbor_mask_lst, p_neighbors, weight_res):
    in_pc_pad = np.asarray(in_pc_pad)
    p = Params(pts=PTS, n_cores=in_pc_pad.shape[0])
    in_maps = host_prep(
        p, in_pc_pad, neighbor_id_lstlst, neighbor_mask_lst, p_neighbors, weight_res
    )
    nc = get_nc(p)
    res = run_bass_kernel_spmd(nc, in_maps, core_ids=list(range(p.n_cores)))
    return assemble(p, res.results)


# revision 14
# speedup vs baseline: 3.4320x; 1.2496x over previous
"""Trainium2 Bass kernel for nn_Pooling_Layer (GNN message-passing pooling).

Math (per batch element b):
    x = in_pc_pad[b] @ weight_res.T               # (N+1, 64) -> (N+1, 128) projection
    w = |p_neighbors| * mask; w /= w.sum(-1)+1e-8 # (P, 32) pooling weights
    out[b, p] = sum_m w[p, m] * x[id[p, m]]       # gather + weighted pool

We reorder: pool first in C_IN=64 space (gather is half the bytes), then
project pooled (P, 64) @ weight_res.T.  Normalization (divide by the weight
sum) is folded into the PSUM->SBUF copy after the projection.

Sharding: points are sharded across the 8 cores (1250 points each); every
core handles ALL batches for its points.  The gather table holds row PAIRS,
batch-interleaved, in bf16: xi[k] = [row 2k: b0..b7 x 64ch | row 2k+1:
b0..b7 x 64ch] (2KB rows).  Pairs keep the SWDGE gather indices int16-safe
(idx = id >> 1 <= 20000); one descriptor serves all 8 batches at a
DMA-efficient 2KB.  bf16 halves HBM gather traffic vs f32; the tolerance
(2e-2) dwarfs bf16 rounding (~0.5%).

Pooling runs on the TensorEngine: per 128-point tile, 64 accumulating bf16
matmuls (32 windows x even/odd half) into one (128 pts, 8b*64ch) PSUM bank.
lhsT is a block-diagonal weight matrix with a FIXED sparsity structure:
window w (slots = partitions: slot 32q+m = neighbor m of point 4w+q) puts
weight at [32q+m, 4w+q].  Even-half weights are |pn|*mask*(1-parity), odd
|pn|*mask*parity, so the wrong half of each gathered pair contributes 0.
The nonzero positions are identical for every tile, so the bd buffers are
zeroed once and only the values are rewritten per tile (tiny strided
copies).

Then per tile: 4 PE transposes (128pts, 2 batches*64ch) -> (128ch, 128pts),
8 projection matmuls lhsT=pooled^T (64,128) rhs=weight_res^T (64,128), and
the per-point 1/denom scale on the PSUM->SBUF copy.  Output is bf16,
upcast and re-assembled on the host.
"""

import numpy as np
import ml_dtypes

import concourse.bass as bass
import concourse.mybir as mybir
import concourse.tile as tile
from concourse import bacc, library_config
from concourse.bass_utils import run_bass_kernel_spmd

F32 = mybir.dt.float32
BF16 = mybir.dt.bfloat16
I16 = mybir.dt.int16

MAXN = 32
CIN = 64
COUT = 128
B = 8
IN_ROWS = 40001          # in_pc_pad rows (incl. pad row)
NPAIRS = 20001           # row pairs (rows padded to 40002)
EW = B * CIN             # interleaved single-row width (elements) = 512
PEW = 2 * EW             # pair-row width = 1024 elements (2KB bf16)
PTS = 10000
NWIN = 32                # windows (4-point groups) per 128-point tile
CHW = 8                  # windows per gather call (1024 idx)
NCALL = NWIN // CHW      # gather calls per tile


class Params:
    def __init__(self, pts=PTS, n_cores=8):
        self.pts = pts
        self.n_cores = n_cores
        self.cpts = pts // n_cores            # points per core (1250)
        self.ntl = (self.cpts + 127) // 128   # 128-point tiles per core (10)
        self.cpts_pad = self.ntl * 128        # 1280


def build_nc(p: Params):
    nc = bacc.Bacc(
        "TRN2",
        target_bir_lowering=False,
        debug=False,
        num_devices=p.n_cores,
        num_swdge_queues=4,
    )
    NTL = p.ntl
    xi = nc.dram_tensor("xi", [NPAIRS, PEW], BF16, kind="ExternalInput")
    idxw = nc.dram_tensor("idxw", [128, NTL * NCALL * 64], I16, kind="ExternalInput")
    pnT = nc.dram_tensor("pnT", [128, p.cpts_pad], F32, kind="ExternalInput")
    maskT = nc.dram_tensor("maskT", [128, p.cpts_pad], F32, kind="ExternalInput")
    parT = nc.dram_tensor("parT", [128, p.cpts_pad], F32, kind="ExternalInput")
    pnN = nc.dram_tensor("pnN", [p.cpts_pad, MAXN], F32, kind="ExternalInput")
    maskN = nc.dram_tensor("maskN", [p.cpts_pad, MAXN], F32, kind="ExternalInput")
    wres = nc.dram_tensor("wres", [COUT, CIN], F32, kind="ExternalInput")
    ident = nc.dram_tensor("ident", [128, 128], F32, kind="ExternalInput")
    out = nc.dram_tensor("out", [B * p.cpts_pad, COUT], BF16, kind="ExternalOutput")

    with tile.TileContext(nc) as tc:
        with (
            tc.tile_pool(name="const", bufs=1) as constp,
            tc.tile_pool(name="prep", bufs=1) as prep,
            tc.tile_pool(name="gather", bufs=6) as gp,
            tc.tile_pool(name="work", bufs=2) as wk,
            tc.tile_pool(name="psP", bufs=2, space="PSUM") as psP,
            tc.tile_pool(name="psT", bufs=2, space="PSUM") as psT,
            tc.tile_pool(name="psO", bufs=2, space="PSUM") as psO,
        ):
            nc.gpsimd.load_library(library_config.mlp)

            # ---- constants ----
            identity = constp.tile([128, 128], F32)
            nc.sync.dma_start(out=identity[:], in_=ident[:])
            wres_sb = constp.tile([COUT, CIN], F32)
            nc.sync.dma_start(out=wres_sb[:], in_=wres[:])
            psw = psT.tile([CIN, COUT], F32, tag="psTt")
            nc.tensor.transpose(out=psw[:], in_=wres_sb[:], identity=identity[:])
            # [i, o] = wres[o, i], replicated into both 64-partition halves so
            # the projection matmul's rhs base partition matches lhsT's
            wresTb = constp.tile([128, COUT], BF16)
            nc.vector.tensor_copy(out=wresTb[0:CIN, :], in_=psw[:])
            nc.vector.tensor_copy(out=wresTb[CIN : 2 * CIN, :], in_=psw[:])

            # idx loaded per tile so the first gather starts immediately
            idx_sb = constp.tile([128, NTL * NCALL * 64], I16)
            for t in range(NTL):
                c0 = t * NCALL * 64
                nc.sync.dma_start(
                    out=idx_sb[:, c0 : c0 + NCALL * 64],
                    in_=idxw[:, c0 : c0 + NCALL * 64],
                )

            # ---- per-point reciprocal denominators: recip[p, t] ----
            prodN = prep.tile([128, NTL * MAXN], F32)
            nc.sync.dma_start(
                out=prodN[:].rearrange("p (t m) -> p t m", m=MAXN),
                in_=pnN[:].rearrange("(t p) m -> p t m", p=128),
            )
            maskN_sb = prep.tile([128, NTL * MAXN], F32)
            nc.sync.dma_start(
                out=maskN_sb[:].rearrange("p (t m) -> p t m", m=MAXN),
                in_=maskN[:].rearrange("(t p) m -> p t m", p=128),
            )
            nc.vector.tensor_tensor(
                out=prodN[:], in0=prodN[:], in1=maskN_sb[:], op=mybir.AluOpType.mult
            )
            denom = constp.tile([128, NTL], F32)
            nc.vector.tensor_reduce(
                out=denom[:],
                in_=prodN[:].rearrange("p (t m) -> p t m", m=MAXN),
                op=mybir.AluOpType.add,
                axis=mybir.AxisListType.X,
                apply_absolute_value=True,
            )
            nc.vector.tensor_scalar_add(denom[:], denom[:], 1e-8)
            recip = constp.tile([128, NTL], F32)
            nc.vector.reciprocal(out=recip[:], in_=denom[:])

            # ---- pooling weights in (32q+m, pt) layout ----
            # wsel0 = |pn|*mask*(1-par)   (even half)
            # wsel1 = |pn|*mask*par       (odd half)
            pnT_sb = prep.tile([128, p.cpts_pad], F32)
            maskT_sb = prep.tile([128, p.cpts_pad], F32)
            parT_sb = prep.tile([128, p.cpts_pad], F32)
            nc.sync.dma_start(out=pnT_sb[:], in_=pnT[:])
            nc.sync.dma_start(out=maskT_sb[:], in_=maskT[:])
            nc.sync.dma_start(out=parT_sb[:], in_=parT[:])
            wsel0 = prep.tile([128, p.cpts_pad], F32)
            wsel1 = prep.tile([128, p.cpts_pad], F32)
            nc.scalar.activation(
                out=wsel0[:], in_=pnT_sb[:], func=mybir.ActivationFunctionType.Abs
            )
            nc.vector.tensor_tensor(
                out=wsel0[:], in0=wsel0[:], in1=maskT_sb[:], op=mybir.AluOpType.mult
            )
            nc.vector.tensor_tensor(
                out=wsel1[:], in0=wsel0[:], in1=parT_sb[:], op=mybir.AluOpType.mult
            )
            nc.vector.tensor_tensor(
                out=wsel0[:], in0=wsel0[:], in1=wsel1[:], op=mybir.AluOpType.subtract
            )

            # ---- block-diag weight buffers: fixed sparsity, zeroed once ----
            BDW = NWIN * 132  # 4224: bd[s, 132w + q] == lhsT col 4w+q of window w
            bd_bufs = []      # [t%2][half] ping-pong pairs
            for i in range(2):
                pair = []
                for half in range(2):
                    bdt = constp.tile([128, BDW], BF16, tag=f"bd{i}h{half}")
                    nc.vector.memset(bdt[:], 0.0)
                    pair.append(bdt)
                bd_bufs.append(pair)

            # ---- main loop over 128-point tiles ----
            for t in range(NTL):
                # windows with at least one real (non-pad) point
                real_pts = min(128, p.cpts - t * 128)
                nwin_t = (real_pts + 3) // 4
                # scatter this tile's weights onto the fixed block-diag slots
                bde, bdo = bd_bufs[t % 2]
                for bd, src in ((bde, wsel0), (bdo, wsel1)):
                    bdv = bd[:].rearrange("p (w c) -> p w c", c=132)
                    sv = src[:, t * 128 : (t + 1) * 128].rearrange(
                        "p (w four) -> p w four", four=4
                    )
                    for q in range(4):
                        nc.vector.tensor_copy(
                            out=bdv[32 * q : 32 * q + 32, :, q],
                            in_=sv[32 * q : 32 * q + 32, :, q],
                        )

                # gather + pool in chunks of 8 windows (1024 idx per call)
                ps = psP.tile([128, EW], F32, tag="ps")
                for c in range((nwin_t + CHW - 1) // CHW):
                    nw_c = min(CHW, nwin_t - c * CHW)
                    g = gp.tile([128, CHW * PEW], BF16, tag="g")
                    call = t * NCALL + c
                    nc.gpsimd.dma_gather(
                        g[:, : nw_c * PEW].rearrange("p (v e) -> p v e", e=PEW),
                        xi[:],
                        idx_sb[:, call * 64 : call * 64 + nw_c * 8],
                        nw_c * 128,
                        nw_c * 128,
                        PEW,
                        queue_num=call % 4,
                    )
                    for v in range(nw_c):
                        w = c * CHW + v
                        for half, bd in ((0, bde), (1, bdo)):
                            nc.tensor.matmul(
                                out=ps[:],
                                lhsT=bd[:, w * 128 : w * 128 + 128],
                                rhs=g[
                                    :,
                                    v * PEW + half * EW : v * PEW + (half + 1) * EW,
                                ],
                                start=(w == 0 and half == 0),
                                stop=(w == nwin_t - 1 and half == 1),
                            )
                pooled = wk.tile([128, EW], F32, tag="pooled")
                nc.scalar.copy(out=pooled[:], in_=ps[:])

                # transpose 2-batch blocks, project, scale by 1/denom, store
                for k in range(4):
                    pst = psT.tile([128, 128], F32, tag="psTt")
                    nc.tensor.transpose(
                        out=pst[:],
                        in_=pooled[:, k * 128 : (k + 1) * 128],
                        identity=identity[:],
                    )
                    poolTb = wk.tile([128, 128], BF16, tag="poolTb")
                    nc.vector.tensor_copy(out=poolTb[:], in_=pst[:])
                    for h in range(2):
                        b = 2 * k + h
                        pso = psO.tile([128, COUT], F32, tag="psO")
                        nc.tensor.matmul(
                            out=pso[:],
                            lhsT=poolTb[64 * h : 64 * h + 64, :],
                            rhs=wresTb[64 * h : 64 * h + 64, :],
                            start=True,
                            stop=True,
                        )
                        outP = wk.tile([128, COUT], BF16, tag="outP")
                        nc.vector.tensor_scalar_mul(
                            outP[:], pso[:], recip[:, t : t + 1]
                        )
                        r0 = b * p.cpts_pad + t * 128
                        nc.sync.dma_start(out=out[r0 : r0 + 128, :], in_=outP[:])
    nc.compile()
    return nc


def host_prep(p: Params, in_pc_pad, ids, mask, pn, wres):
    """Per-core input maps.  Host work is layout marshalling only."""
    ids = np.asarray(ids).astype(np.int64)
    pn = np.asarray(pn, dtype=np.float32)
    mask = np.asarray(mask, dtype=np.float32)
    wres = np.asarray(wres, dtype=np.float32)
    x = np.asarray(in_pc_pad, dtype=np.float32)          # (B, 40001, 64)

    # pair table: xi[k] = [row 2k all batches | row 2k+1 all batches], bf16
    xp = np.concatenate([x, np.zeros((B, 1, CIN), np.float32)], axis=1)
    xi = np.ascontiguousarray(
        xp.transpose(1, 0, 2).reshape(2 * NPAIRS, EW).reshape(NPAIRS, PEW)
    ).astype(ml_dtypes.bfloat16)
    ident = np.eye(128, dtype=np.float32)

    in_maps = []
    for c in range(p.n_cores):
        lo = c * p.cpts

        def pad_pts(a, dtype):
            o = np.zeros((p.cpts_pad, MAXN), dtype=dtype)
            o[: p.cpts] = a[lo : lo + p.cpts]
            return o

        ids_c = pad_pts(ids, np.int64)
        ids_c[p.cpts :] = 2 * (NPAIRS - 1)               # pad points: valid pair
        pn_c = pad_pts(pn, np.float32)
        mask_c = pad_pts(mask, np.float32)
        par_c = (ids_c & 1).astype(np.float32)
        idx16 = (ids_c >> 1).astype(np.int16)

        # gather stream: tile t, window w, slot s=32q+m -> ids_c[t*128+4w+q, m]>>1
        flat = (
            idx16.reshape(p.ntl, NWIN, 4, MAXN)
            .transpose(0, 1, 2, 3)                       # (t, w, q, m)
            .reshape(p.ntl * NWIN * 128)
        )
        # wrapped-16 layout per 1024-idx call: idx i at [i % 16, i // 16]
        ncalls = p.ntl * NCALL
        idx_w = np.zeros((128, ncalls * 64), np.int16)
        for call in range(ncalls):
            blk = flat[call * 1024 : (call + 1) * 1024].reshape(64, 16).T
            idx_w[:, call * 64 : (call + 1) * 64] = np.tile(blk, (8, 1))

        pnT = np.ascontiguousarray(np.tile(pn_c.T, (4, 1)))      # (128, cpts_pad)
        maskT = np.ascontiguousarray(np.tile(mask_c.T, (4, 1)))
        parT = np.ascontiguousarray(np.tile(par_c.T, (4, 1)))
        in_maps.append(
            {
                "xi": xi,
                "idxw": idx_w,
                "pnT": pnT,
                "maskT": maskT,
                "parT": parT,
                "pnN": pn_c,
                "maskN": mask_c,
                "wres": wres,
                "ident": ident,
            }
        )
    return in_maps


def assemble(p: Params, results):
    out = np.empty((B, p.pts, COUT), np.float32)
    for c in range(p.n_cores):
        got = np.asarray(results[c]["out"], dtype=np.float32).reshape(
            B, p.cpts_pad, COUT
        )
        out[:, c * p.cpts : (c + 1) * p.cpts, :] = got[:, : p.cpts, :]
    return out


_NC_CACHE = {}


def get_nc(p: Params):
    key = (p.pts, p.n_cores)
    if key not in _NC_CACHE:
        _NC_CACHE[key] = build_nc(p)
    return _NC_CACHE[key]


def kernel(in_pc_pad, neighbor_id_lstlst, neighbor_mask_lst, p_neighbors, weight_res):
    in_pc_pad = np.asarray(in_pc_pad)
    p = Params(pts=PTS, n_cores=in_pc_pad.shape[0])
    in_maps = host_prep(
        p, in_pc_pad, neighbor_id_lstlst, neighbor_mask_lst, p_neighbors, weight_res
    )
    nc = get_nc(p)
    res = run_bass_kernel_spmd(nc, in_maps, core_ids=list(range(p.n_cores)))
    return assemble(p, res.results)


# revision 16
# speedup vs baseline: 3.5823x; 1.0438x over previous
"""Trainium2 Bass kernel for nn_Pooling_Layer (GNN message-passing pooling).

Math (per batch element b):
    x = in_pc_pad[b] @ weight_res.T               # (N+1, 64) -> (N+1, 128) projection
    w = |p_neighbors| * mask; w /= w.sum(-1)+1e-8 # (P, 32) pooling weights
    out[b, p] = sum_m w[p, m] * x[id[p, m]]       # gather + weighted pool

We reorder: pool first in C_IN=64 space (gather is half the bytes), then
project pooled (P, 64) @ weight_res.T.  Normalization (divide by the weight
sum) is folded into the PSUM->SBUF copy after the projection.

Sharding: points are sharded across the 8 cores (1250 points each); every
core handles ALL batches for its points.  The gather table holds row PAIRS,
batch-interleaved, in bf16: xi[k] = [row 2k: b0..b7 x 64ch | row 2k+1:
b0..b7 x 64ch] (2KB rows).  Pairs keep the SWDGE gather indices int16-safe
(idx = id >> 1 <= 20000); one descriptor serves all 8 batches at a
DMA-efficient 2KB.  bf16 halves HBM gather traffic vs f32; the tolerance
(2e-2) dwarfs bf16 rounding (~0.5%).

Pooling runs on the TensorEngine: per 128-point tile, 64 accumulating bf16
matmuls (32 windows x even/odd half) into one (128 pts, 8b*64ch) PSUM bank.
lhsT is a block-diagonal weight matrix with a FIXED sparsity structure:
window w (slots = partitions: slot 32q+m = neighbor m of point 4w+q) puts
weight at [32q+m, 4w+q].  Even-half weights are |pn|*mask*(1-parity), odd
|pn|*mask*parity, so the wrong half of each gathered pair contributes 0.
The nonzero positions are identical for every tile, so the bd buffers are
zeroed once and only the values are rewritten per tile (tiny strided
copies).

Then per tile: 4 PE transposes (128pts, 2 batches*64ch) -> (128ch, 128pts),
8 projection matmuls lhsT=pooled^T (64,128) rhs=weight_res^T (64,128), and
the per-point 1/denom scale on the PSUM->SBUF copy.  Output is bf16,
upcast and re-assembled on the host.
"""

import numpy as np
import ml_dtypes

import concourse.bass as bass
import concourse.mybir as mybir
import concourse.tile as tile
from concourse import bacc, library_config
from concourse.bass_utils import run_bass_kernel_spmd

F32 = mybir.dt.float32
BF16 = mybir.dt.bfloat16
I16 = mybir.dt.int16

MAXN = 32
CIN = 64
COUT = 128
B = 8
IN_ROWS = 40001          # in_pc_pad rows (incl. pad row)
NPAIRS = 20001           # row pairs (rows padded to 40002)
EW = B * CIN             # interleaved single-row width (elements) = 512
PEW = 2 * EW             # pair-row width = 1024 elements (2KB bf16)
PTS = 10000
NWIN = 32                # windows (4-point groups) per 128-point tile
CHW = 8                  # windows per gather call (1024 idx)
NCALL = (NWIN + CHW - 1) // CHW   # gather call slots per tile


class Params:
    def __init__(self, pts=PTS, n_cores=8):
        self.pts = pts
        self.n_cores = n_cores
        self.cpts = pts // n_cores            # points per core (1250)
        self.ntl = (self.cpts + 127) // 128   # 128-point tiles per core (10)
        self.cpts_pad = self.ntl * 128        # 1280


def build_nc(p: Params):
    nc = bacc.Bacc(
        "TRN2",
        target_bir_lowering=False,
        debug=False,
        num_devices=p.n_cores,
        num_swdge_queues=4,
    )
    NTL = p.ntl
    xi = nc.dram_tensor("xi", [NPAIRS, PEW], BF16, kind="ExternalInput")
    idxw = nc.dram_tensor("idxw", [128, NTL * NWIN * 8], I16, kind="ExternalInput")
    pnT = nc.dram_tensor("pnT", [128, p.cpts_pad], F32, kind="ExternalInput")
    maskT = nc.dram_tensor("maskT", [128, p.cpts_pad], F32, kind="ExternalInput")
    parT = nc.dram_tensor("parT", [128, p.cpts_pad], F32, kind="ExternalInput")
    pnN = nc.dram_tensor("pnN", [p.cpts_pad, MAXN], F32, kind="ExternalInput")
    maskN = nc.dram_tensor("maskN", [p.cpts_pad, MAXN], F32, kind="ExternalInput")
    wres = nc.dram_tensor("wres", [COUT, CIN], F32, kind="ExternalInput")
    ident = nc.dram_tensor("ident", [128, 128], F32, kind="ExternalInput")
    out = nc.dram_tensor("out", [B * p.cpts_pad, COUT], BF16, kind="ExternalOutput")

    with tile.TileContext(nc) as tc:
        with (
            tc.tile_pool(name="const", bufs=1) as constp,
            tc.tile_pool(name="prep", bufs=1) as prep,
            tc.tile_pool(name="gather", bufs=6) as gp,
            tc.tile_pool(name="work", bufs=2) as wk,
            tc.tile_pool(name="psP", bufs=2, space="PSUM") as psP,
            tc.tile_pool(name="psT", bufs=2, space="PSUM") as psT,
            tc.tile_pool(name="psO", bufs=2, space="PSUM") as psO,
        ):
            nc.gpsimd.load_library(library_config.mlp)

            # ---- constants ----
            identity = constp.tile([128, 128], F32)
            nc.sync.dma_start(out=identity[:], in_=ident[:])
            wres_sb = constp.tile([COUT, CIN], F32)
            nc.sync.dma_start(out=wres_sb[:], in_=wres[:])
            psw = psT.tile([CIN, COUT], F32, tag="psTt")
            nc.tensor.transpose(out=psw[:], in_=wres_sb[:], identity=identity[:])
            # [i, o] = wres[o, i], replicated into both 64-partition halves so
            # the projection matmul's rhs base partition matches lhsT's
            wresTb = constp.tile([128, COUT], BF16)
            nc.vector.tensor_copy(out=wresTb[0:CIN, :], in_=psw[:])
            nc.vector.tensor_copy(out=wresTb[CIN : 2 * CIN, :], in_=psw[:])

            # idx loaded per tile so the first gather starts immediately
            idx_sb = constp.tile([128, NTL * NWIN * 8], I16)
            for t in range(NTL):
                c0 = t * NWIN * 8
                nc.sync.dma_start(
                    out=idx_sb[:, c0 : c0 + NWIN * 8],
                    in_=idxw[:, c0 : c0 + NWIN * 8],
                )

            # ---- per-point reciprocal denominators: recip[p, t] ----
            prodN = prep.tile([128, NTL * MAXN], F32)
            nc.sync.dma_start(
                out=prodN[:].rearrange("p (t m) -> p t m", m=MAXN),
                in_=pnN[:].rearrange("(t p) m -> p t m", p=128),
            )
            maskN_sb = prep.tile([128, NTL * MAXN], F32)
            nc.sync.dma_start(
                out=maskN_sb[:].rearrange("p (t m) -> p t m", m=MAXN),
                in_=maskN[:].rearrange("(t p) m -> p t m", p=128),
            )
            nc.vector.tensor_tensor(
                out=prodN[:], in0=prodN[:], in1=maskN_sb[:], op=mybir.AluOpType.mult
            )
            denom = constp.tile([128, NTL], F32)
            nc.vector.tensor_reduce(
                out=denom[:],
                in_=prodN[:].rearrange("p (t m) -> p t m", m=MAXN),
                op=mybir.AluOpType.add,
                axis=mybir.AxisListType.X,
                apply_absolute_value=True,
            )
            nc.vector.tensor_scalar_add(denom[:], denom[:], 1e-8)
            recip = constp.tile([128, NTL], F32)
            nc.vector.reciprocal(out=recip[:], in_=denom[:])

            # ---- pooling weights in (32q+m, pt) layout ----
            # wsel0 = |pn|*mask*(1-par)   (even half)
            # wsel1 = |pn|*mask*par       (odd half)
            pnT_sb = prep.tile([128, p.cpts_pad], F32)
            maskT_sb = prep.tile([128, p.cpts_pad], F32)
            parT_sb = prep.tile([128, p.cpts_pad], F32)
            nc.sync.dma_start(out=pnT_sb[:], in_=pnT[:])
            nc.sync.dma_start(out=maskT_sb[:], in_=maskT[:])
            nc.sync.dma_start(out=parT_sb[:], in_=parT[:])
            wsel0 = prep.tile([128, p.cpts_pad], F32)
            wsel1 = prep.tile([128, p.cpts_pad], F32)
            nc.scalar.activation(
                out=wsel0[:], in_=pnT_sb[:], func=mybir.ActivationFunctionType.Abs
            )
            nc.vector.tensor_tensor(
                out=wsel0[:], in0=wsel0[:], in1=maskT_sb[:], op=mybir.AluOpType.mult
            )
            nc.vector.tensor_tensor(
                out=wsel1[:], in0=wsel0[:], in1=parT_sb[:], op=mybir.AluOpType.mult
            )
            nc.vector.tensor_tensor(
                out=wsel0[:], in0=wsel0[:], in1=wsel1[:], op=mybir.AluOpType.subtract
            )

            # ---- block-diag weight buffers: fixed sparsity, zeroed once ----
            BDW = NWIN * 132  # 4224: bd[s, 132w + q] == lhsT col 4w+q of window w
            bd_bufs = []      # [t%2][half] ping-pong pairs
            for i in range(2):
                pair = []
                for half in range(2):
                    bdt = constp.tile([128, BDW], BF16, tag=f"bd{i}h{half}")
                    nc.vector.memset(bdt[:], 0.0)
                    pair.append(bdt)
                bd_bufs.append(pair)

            # ---- main loop over 128-point tiles ----
            for t in range(NTL):
                # windows with at least one real (non-pad) point
                real_pts = min(128, p.cpts - t * 128)
                nwin_t = (real_pts + 3) // 4
                # scatter this tile's weights onto the fixed block-diag slots
                bde, bdo = bd_bufs[t % 2]
                for bd, src in ((bde, wsel0), (bdo, wsel1)):
                    bdv = bd[:].rearrange("p (w c) -> p w c", c=132)
                    sv = src[:, t * 128 : (t + 1) * 128].rearrange(
                        "p (w four) -> p w four", four=4
                    )
                    for q in range(4):
                        nc.vector.tensor_copy(
                            out=bdv[32 * q : 32 * q + 32, :, q],
                            in_=sv[32 * q : 32 * q + 32, :, q],
                        )

                # gather + pool in chunks of 8 windows (1024 idx per call)
                ps = psP.tile([128, EW], F32, tag="ps")
                for c in range((nwin_t + CHW - 1) // CHW):
                    nw_c = min(CHW, nwin_t - c * CHW)
                    g = gp.tile([128, CHW * PEW], BF16, tag="g")
                    call = t * NCALL + c
                    col0 = t * NWIN * 8 + c * CHW * 8
                    nc.gpsimd.dma_gather(
                        g[:, : nw_c * PEW].rearrange("p (v e) -> p v e", e=PEW),
                        xi[:],
                        idx_sb[:, col0 : col0 + nw_c * 8],
                        nw_c * 128,
                        nw_c * 128,
                        PEW,
                        queue_num=call % 4,
                    )
                    for v in range(nw_c):
                        w = c * CHW + v
                        for half, bd in ((0, bde), (1, bdo)):
                            nc.tensor.matmul(
                                out=ps[:],
                                lhsT=bd[:, w * 128 : w * 128 + 128],
                                rhs=g[
                                    :,
                                    v * PEW + half * EW : v * PEW + (half + 1) * EW,
                                ],
                                start=(w == 0 and half == 0),
                                stop=(w == nwin_t - 1 and half == 1),
                            )
                pooled = wk.tile([128, EW], F32, tag="pooled")
                nc.scalar.copy(out=pooled[:], in_=ps[:])

                # transpose 2-batch blocks, project, scale by 1/denom, store
                for k in range(4):
                    pst = psT.tile([128, 128], F32, tag="psTt")
                    nc.tensor.transpose(
                        out=pst[:],
                        in_=pooled[:, k * 128 : (k + 1) * 128],
                        identity=identity[:],
                    )
                    poolTb = wk.tile([128, 128], BF16, tag="poolTb")
                    nc.vector.tensor_copy(out=poolTb[:], in_=pst[:])
                    for h in range(2):
                        b = 2 * k + h
                        pso = psO.tile([128, COUT], F32, tag="psO")
                        nc.tensor.matmul(
                            out=pso[:],
                            lhsT=poolTb[64 * h : 64 * h + 64, :],
                            rhs=wresTb[64 * h : 64 * h + 64, :],
                            start=True,
                            stop=True,
                        )
                        outP = wk.tile([128, COUT], BF16, tag="outP")
                        nc.vector.tensor_scalar_mul(
                            outP[:], pso[:], recip[:, t : t + 1]
                        )
                        r0 = b * p.cpts_pad + t * 128
                        nc.sync.dma_start(out=out[r0 : r0 + 128, :], in_=outP[:])
    nc.compile()
    return nc


def host_prep(p: Params, in_pc_pad, ids, mask, pn, wres):
    """Per-core input maps.  Host work is layout marshalling only."""
    ids = np.asarray(ids).astype(np.int64)
    pn = np.asarray(pn, dtype=np.float32)
    mask = np.asarray(mask, dtype=np.float32)
    wres = np.asarray(wres, dtype=np.float32)
    x = np.asarray(in_pc_pad, dtype=np.float32)          # (B, 40001, 64)

    # pair table: xi[k] = [row 2k all batches | row 2k+1 all batches], bf16
    xp = np.concatenate([x, np.zeros((B, 1, CIN), np.float32)], axis=1)
    xi = np.ascontiguousarray(
        xp.transpose(1, 0, 2).reshape(2 * NPAIRS, EW).reshape(NPAIRS, PEW)
    ).astype(ml_dtypes.bfloat16)
    ident = np.eye(128, dtype=np.float32)

    in_maps = []
    for c in range(p.n_cores):
        lo = c * p.cpts

        def pad_pts(a, dtype):
            o = np.zeros((p.cpts_pad, MAXN), dtype=dtype)
            o[: p.cpts] = a[lo : lo + p.cpts]
            return o

        ids_c = pad_pts(ids, np.int64)
        ids_c[p.cpts :] = 2 * (NPAIRS - 1)               # pad points: valid pair
        pn_c = pad_pts(pn, np.float32)
        mask_c = pad_pts(mask, np.float32)
        par_c = (ids_c & 1).astype(np.float32)
        idx16 = (ids_c >> 1).astype(np.int16)

        # gather stream: tile t, window w, slot s=32q+m -> ids_c[t*128+4w+q, m]>>1
        flat = (
            idx16.reshape(p.ntl, NWIN, 4, MAXN)
            .transpose(0, 1, 2, 3)                       # (t, w, q, m)
            .reshape(p.ntl * NWIN * 128)
        )
        # wrapped-16 layout per call: idx i at [i % 16, i // 16]
        idx_w = np.zeros((128, p.ntl * NWIN * 8), np.int16)
        for t in range(p.ntl):
            for c in range(NCALL):
                w0 = c * CHW
                nwn = min(CHW, NWIN - w0)
                blk_flat = flat[t * 4096 + w0 * 128 : t * 4096 + (w0 + nwn) * 128]
                blk = blk_flat.reshape(nwn * 8, 16).T
                col0 = t * NWIN * 8 + c * CHW * 8
                idx_w[:, col0 : col0 + nwn * 8] = np.tile(blk, (8, 1))

        pnT = np.ascontiguousarray(np.tile(pn_c.T, (4, 1)))      # (128, cpts_pad)
        maskT = np.ascontiguousarray(np.tile(mask_c.T, (4, 1)))
        parT = np.ascontiguousarray(np.tile(par_c.T, (4, 1)))
        in_maps.append(
            {
                "xi": xi,
                "idxw": idx_w,
                "pnT": pnT,
                "maskT": maskT,
                "parT": parT,
                "pnN": pn_c,
                "maskN": mask_c,
                "wres": wres,
                "ident": ident,
            }
        )
    return in_maps


def assemble(p: Params, results):
    out = np.empty((B, p.pts, COUT), np.float32)
    for c in range(p.n_cores):
        got = np.asarray(results[c]["out"], dtype=np.float32).reshape(
            B, p.cpts_pad, COUT
        )
        out[:, c * p.cpts : (c + 1) * p.cpts, :] = got[:, : p.cpts, :]
    return out


_NC_CACHE = {}


def get_nc(p: Params):
    key = (p.pts, p.n_cores)
    if key not in _NC_CACHE:
        _NC_CACHE[key] = build_nc(p)
    return _NC_CACHE[key]


def kernel(in_pc_pad, neighbor_id_lstlst, neighbor_mask_lst, p_neighbors, weight_res):
    in_pc_pad = np.asarray(in_pc_pad)
    p = Params(pts=PTS, n_cores=in_pc_pad.shape[0])
    in_maps = host_prep(
        p, in_pc_pad, neighbor_id_lstlst, neighbor_mask_lst, p_neighbors, weight_res
    )
    nc = get_nc(p)
    res = run_bass_kernel_spmd(nc, in_maps, core_ids=list(range(p.n_cores)))
    return assemble(p, res.results)
